# revision 3
# baseline (speedup 1.0000x reference)
"""Trainium2 Bass kernel for nn_Decoder (dense MLP).

Computes out = relu(V @ W1 + b1) @ W2 + b2 for V [262144, 1024],
W1 [1024, 128], W2 [128, 4].

Strategy
--------
Data-parallel over 8 NeuronCores: V is sharded along rows (32768 rows per
core); the small weights are replicated. Each core's V shard is transposed
on the host to [1024, 32768] fp16 so the contraction dim (1024) lands on
SBUF partitions with fully contiguous DMA loads — no on-chip transposes.

Single-pass fp16 (~3e-4 rel err, gate is 2e-2) instead of the previous
3-pass hi/lo split — 3x less PE work. Layer 1 computes h.T = W1.T @ V.T
via PSUM-accumulated matmuls; the loop is k-major over S=4 PSUM banks so
each W1 k-chunk's weight load is amortized over 4 matmuls. ReLU(+b1) on
the scalar engine reads PSUM and emits fp16; layer 2 (out.T = W2.T @ h.T)
runs on the tensor engine; b2 is added on the vector engine and results
are stored per 2048-column superchunk.

Expected regime: HBM-bound (67 MB fp16 V per core at ~358 GB/s ≈ 190 us).
"""

import os
import sys

import numpy as np

for _p in ("/opt/trn_rl_repo", "/root/.axon_site/_ro/trn_rl_repo"):
    if os.path.isdir(_p) and _p not in sys.path:
        sys.path.insert(0, _p)

import concourse.bass as bass
import concourse.mybir as mybir
import concourse.tile as tile
from concourse import bacc
from concourse.bass_utils import run_bass_kernel_spmd

NCORES = 8
NN = 262144
IN_DIM = 1024
HIDDEN = 128
OUT_DIM = 4
R = NN // NCORES  # rows per core

P = 128            # SBUF partitions
KC = IN_DIM // P   # 8 k-chunks
CHUNK = 512        # columns per PSUM accumulation tile (one PSUM bank)
S = 4              # chunks per superchunk (live PSUM accumulator banks)
SGROUP = S * CHUNK # 2048 columns per DMA group / superchunk
DATA_BUFS = 3      # prefetch depth for V superchunk tiles

_last_results = None  # exposed for test harness (exec_time_ns etc.)
MODE = "f16"


def build_nc(rows=R):
    """Build the SPMD Bass program for one core."""
    f32 = mybir.dt.float32
    f16 = mybir.dt.float16

    nc = bacc.Bacc("TRN2")

    vt_d = nc.declare_dram_parameter("VT", [IN_DIM, rows], f16, isOutput=False)
    w1_d = nc.declare_dram_parameter("W1T", [IN_DIM, HIDDEN], f16, isOutput=False)
    b1_d = nc.declare_dram_parameter("B1", [HIDDEN, 1], f32, isOutput=False)
    w2_d = nc.declare_dram_parameter("W2T", [HIDDEN, OUT_DIM], f16, isOutput=False)
    b2_d = nc.declare_dram_parameter("B2", [OUT_DIM, 1], f32, isOutput=False)
    out_d = nc.declare_dram_parameter("OUT", [OUT_DIM, rows], f32, isOutput=True)

    nsc = rows // SGROUP

    with tile.TileContext(nc) as tc:
        with (
            tc.tile_pool(name="const", bufs=1) as cpool,
            tc.tile_pool(name="data", bufs=DATA_BUFS) as dpool,
            tc.tile_pool(name="work", bufs=3) as wpool,
            tc.tile_pool(name="psum1", bufs=4, space="PSUM") as ppool,
            tc.tile_pool(name="psum2", bufs=2, space="PSUM") as opool,
        ):
            # --- constants (loaded once) ---
            w1_sb = cpool.tile([P, KC, HIDDEN], f16)
            nc.sync.dma_start(w1_sb[:], w1_d[:].rearrange("(c p) h -> p c h", p=P))
            b1_sb = cpool.tile([HIDDEN, 1], f32)
            nc.sync.dma_start(b1_sb[:], b1_d[:])
            w2_sb = cpool.tile([HIDDEN, OUT_DIM], f16)
            nc.sync.dma_start(w2_sb[:], w2_d[:])
            b2_sb = cpool.tile([OUT_DIM, 1], f32)
            nc.sync.dma_start(b2_sb[:], b2_d[:])

            vt_view = vt_d[:].rearrange("(c p) (g n) -> g p c n", p=P, n=SGROUP)
            out_view = out_d[:].rearrange("o (g n) -> g o n", n=SGROUP)

            for g in range(nsc):
                v = dpool.tile([P, KC, SGROUP], f16, tag="v")
                if g == 0:
                    # split the first superchunk per-chunk so PE starts early
                    for j in range(S):
                        slv = slice(j * CHUNK, (j + 1) * CHUNK)
                        nc.sync.dma_start(v[:, :, slv], vt_view[g][:, :, slv])
                else:
                    nc.sync.dma_start(v[:], vt_view[g])

                # layer 1: k-major so each W1 k-chunk stays stationary
                # across S matmuls (amortizes the 128-col weight load)
                ps = [
                    ppool.tile([HIDDEN, CHUNK], f32, tag="ps", name=f"ps{j}")
                    for j in range(S)
                ]
                for c in range(KC):
                    for j in range(S):
                        nc.tensor.matmul(
                            ps[j][:],
                            w1_sb[:, c, :],
                            v[:, c, j * CHUNK : (j + 1) * CHUNK],
                            start=(c == 0),
                            stop=(c == KC - 1),
                        )

                o_sb = wpool.tile([OUT_DIM, SGROUP], f32, tag="o")
                for j in range(S):
                    hh = wpool.tile([HIDDEN, CHUNK], f16, tag="hh")
                    nc.scalar.activation(
                        hh[:], ps[j][:],
                        mybir.ActivationFunctionType.Relu,
                        bias=b1_sb[:],
                    )
                    po = opool.tile([OUT_DIM, CHUNK], f32, tag="po")
                    nc.tensor.matmul(po[:], w2_sb[:], hh[:], start=True, stop=True)
                    nc.vector.tensor_scalar_add(
                        o_sb[:, j * CHUNK : (j + 1) * CHUNK], po[:], b2_sb[:]
                    )

                nc.scalar.dma_start(out_view[g], o_sb[:])

    return nc


def kernel(V, W1, b1, W2, b2):
    global _last_results

    V = np.asarray(V, dtype=np.float32)
    W1 = np.asarray(W1, dtype=np.float32)
    b1 = np.asarray(b1, dtype=np.float32)
    W2 = np.asarray(W2, dtype=np.float32)
    b2 = np.asarray(b2, dtype=np.float32)

    common = {
        "W1T": W1.astype(np.float16),
        "W2T": np.ascontiguousarray(W2).astype(np.float16),
        "B1": np.ascontiguousarray(b1.reshape(HIDDEN, 1)),
        "B2": np.ascontiguousarray(b2.reshape(OUT_DIM, 1)),
    }

    in_maps = []
    for c in range(NCORES):
        shard = V[c * R : (c + 1) * R]  # [R, IN_DIM]
        m = {"VT": np.ascontiguousarray(shard.T.astype(np.float16))}
        m.update(common)
        in_maps.append(m)

    nc = build_nc(R)
    nc.finalize()
    res = run_bass_kernel_spmd(nc, in_maps, list(range(NCORES)))
    _last_results = res

    out = np.concatenate(
        [np.asarray(r["OUT"]).T for r in res.results], axis=0
    ).astype(np.float32)
    return out
